# revision 1
# baseline (speedup 1.0000x reference)
# DigitCaps dynamic-routing kernel for 8 Trainium2 NeuronCores.
#
# Sharding: the prev-layer node axis P=6272 is split across the 8 cores
# (784 nodes each).  Per core, both W-slices and u-slices live in SBUF in
# bf16 for the whole kernel; every routing sweep recomputes u_hat tiles
# on the PE from SBUF instead of streaming a 514MB u_hat through HBM.
# The only cross-core traffic is three small AllReduces of the per-core
# partial sums s_raw[n,b,o] (+ softmax denominators Z[n,b]).
#
# Layout glossary (per core, local p in [0,784)):
#   NB layout: partitions = (ns, bs) = 8 caps x 16 batch  (per n-group g, b-half h)
#   P  layout: partitions = local p (7 chunks of 128, last chunk 16 valid)
#   w2   [5g][128=(ns,o)][784p][8i]   rhs of the wv matmul (streamed from HBM)
#   wp2  [128=p][7ch][40n][128=(i,o)] lhsT of the s matmul
#   up2  [128=p][7ch][8i][32b]        rhs of the s matmul (iter 1) / cu fold input
#   urep [2h][128=(ns,bs)][784p][8i]  u replicated over ns, for the a-pass fold
import os
import numpy as np
import ml_dtypes

import concourse.bass as bass
import concourse.bacc as bacc
import concourse.tile as tile
import concourse.mybir as mybir
from concourse.bass_utils import run_bass_kernel_spmd

BF16 = mybir.dt.bfloat16
F32 = mybir.dt.float32
AX = mybir.AxisListType
ALU = mybir.AluOpType
ACTF = mybir.ActivationFunctionType

N, P, I, O, B = 40, 6272, 8, 16, 32
NC = 8
PL = P // NC          # 784 local nodes
NG = 5                # n-groups of 8
BH = 2                # b-halves of 16
CH = 7                # p-chunks of 128 (last has 16 valid rows)
PPAD = CH * 128       # 896
NB_TILES = NG * BH    # 10 (g, h) tiles; tile t = 2*g + h
CCLEN = O * N * B + N * B  # 20480 s_raw + 1280 Z


def _build_program(for_sim=False):
    nc = bacc.Bacc("TRN2", target_bir_lowering=False, debug=False)

    w2 = nc.dram_tensor("w2", [NG, 128, PL, I], BF16, kind="ExternalInput")
    wp2 = nc.dram_tensor("wp2", [128, CH, N, 128], BF16, kind="ExternalInput")
    up2 = nc.dram_tensor("up2", [128, CH, I, B], BF16, kind="ExternalInput")
    urep = nc.dram_tensor("urep", [BH, 128, PL, I], BF16, kind="ExternalInput")
    bdmask = nc.dram_tensor("bdmask", [128, 128], BF16, kind="ExternalInput")
    selio = nc.dram_tensor("selio", [128, I, 16], F32, kind="ExternalInput")
    id128b = nc.dram_tensor("id128b", [128, 128], BF16, kind="ExternalInput")
    id16f = nc.dram_tensor("id16f", [16, 16], F32, kind="ExternalInput")
    vout = nc.dram_tensor("vout", [N, B, O], F32, kind="ExternalOutput")

    with tile.TileContext(nc) as tc:
        with (
            tc.tile_pool(name="res", bufs=1) as res,       # whole-kernel residents
            tc.tile_pool(name="w2s", bufs=3) as w2s,       # streamed w2 pieces
            tc.tile_pool(name="work", bufs=3) as work,     # wv/t chunk tiles
            tc.tile_pool(name="atile", bufs=2) as atile,   # a / b2 tiles
            tc.tile_pool(name="cupool", bufs=2) as cupool,
            tc.tile_pool(name="sm", bufs=1) as sm,         # small per-tile stats
            tc.tile_pool(name="ps_s", bufs=1, space="PSUM") as ps_s,
            tc.tile_pool(name="ps_wv", bufs=2, space="PSUM") as ps_wv,
            tc.tile_pool(name="ps_m", bufs=2, space="PSUM") as ps_m,
            tc.tile_pool(name="dram", bufs=2, space="DRAM") as dram,
        ):
            # ---- residents ----
            sb_wp2 = res.tile([128, CH, N, 128], BF16)
            nc.sync.dma_start(out=sb_wp2, in_=wp2[:])
            sb_up2 = res.tile([128, CH, I, B], BF16)
            nc.sync.dma_start(out=sb_up2, in_=up2[:])
            sb_urep0 = res.tile([128, PL, I], BF16)
            nc.sync.dma_start(out=sb_urep0, in_=urep[0])
            sb_urep1 = res.tile([128, PL, I], BF16)
            nc.sync.dma_start(out=sb_urep1, in_=urep[1])
            sb_urep = [sb_urep0, sb_urep1]
            sb_mask = res.tile([128, 128], BF16)
            nc.sync.dma_start(out=sb_mask, in_=bdmask[:])
            sb_sel = res.tile([128, I, 16], F32)
            nc.sync.dma_start(out=sb_sel, in_=selio[:])
            sb_id128b = res.tile([128, 128], BF16)
            nc.sync.dma_start(out=sb_id128b, in_=id128b[:])
            sb_id16f = res.tile([16, 16], F32)
            nc.sync.dma_start(out=sb_id16f, in_=id16f[:])

            a1_dram = dram.tile([NB_TILES, 128, PL], F32, tag="a1", bufs=1)

            sb_eP2 = res.tile([128, CH, N, B], BF16)
            nc.vector.memset(sb_eP2, 0.0)
            sb_Z = res.tile([128, NB_TILES], F32)
            # iteration-1 "Z": AllReduce over 8 cores must sum to P (uniform c)
            nc.vector.memset(sb_Z, float(P) / NC)
            # s_raw columns in (g, h, ns, bs) order: col 128*(2g+h) + 16*ns + bs
            sb_sraw = res.tile([16, NG, BH, 8, 16], F32)
            sb_sglob = res.tile([16, N * B], F32)
            sb_Zg = res.tile([128, NB_TILES], F32)
            sb_vT = res.tile([16, N * B], BF16)
            sb_vT8 = res.tile([128, N * B], BF16)
            sb_fac = res.tile([128, NB_TILES], F32)
            sb_ss = res.tile([128, NB_TILES], F32)

            junk_sb = res.tile([128, 1], F32)

            def pe_touch(*aps):
                """Tiny matmuls that consume pending semaphores on the PE
                queue (the lowered LDWEIGHTS has a single sync-wait slot, so
                real matmuls may carry at most one fresh dependency)."""
                for ap in aps:
                    j = ps_s.tile([1, 1], F32, tag="ps_s", name="jnk")
                    nc.tensor.matmul(j, ap, ap, start=True, stop=True)

            def act_touch(ap):
                nc.scalar.copy(out=junk_sb[0 : ap.shape[0], :], in_=ap)

            def s_pass(it):
                """Partial s_raw[n,b,o] = sum_{p local,i} cu * W, AllReduce,
                squash.  it=1 uses cu = u (uniform c), else cu = eP2 * up2."""
                for jb in range(NG * 2):
                    g_, nhalf = jb // 2, jb % 2
                    psum_s = ps_s.tile([128, 4, 256], F32, tag="ps_s", bufs=2)
                    for nn in range(4):
                        n = 4 * jb + nn
                        if it == 1:
                            cu = sb_up2
                        else:
                            cu = cupool.tile([128, CH, I, B], BF16, tag="cu")
                            e_sl = bass.AP(
                                tensor=sb_eP2.tensor,
                                offset=sb_eP2.offset + n * B,
                                ap=[sb_eP2.ap[0], [N * B, CH], [0, I], [1, B]],
                            )
                            nc.vector.tensor_tensor(
                                out=cu, in0=sb_up2, in1=e_sl, op=ALU.mult
                            )
                        cu_flat = cu.rearrange("q c i b -> q c (i b)")
                        for ch in range(CH):
                            nc.tensor.matmul(
                                psum_s[:, nn, :],
                                sb_wp2[:, ch, n, :],
                                cu_flat[:, ch, :],
                                start=(ch == 0),
                                stop=(ch == CH - 1),
                            )
                    # extract s[o,n,b] = sum_i psum[(i,o), nn, (i,b)]:
                    # copy the whole block to SBUF, then 8 accumulating
                    # selector matmuls: sel[:,i,:] keeps only rows (i,o)
                    # while the rhs free-offset slides to the i-th b-slice.
                    Ssb = sm.tile([128, I, 4, B], F32, tag="S2", bufs=2)
                    nc.scalar.copy(
                        out=Ssb.rearrange("q i n b -> q n i b"),
                        in_=psum_s.rearrange("q n (i b) -> q n i b", i=I),
                    )
                    Ssb_flat = Ssb.rearrange("q i n b -> q i (n b)")
                    sel_ps = ps_m.tile([16, 4, B], F32, tag="m")
                    for i in range(I):
                        nc.tensor.matmul(
                            sel_ps,
                            sb_sel[:, i, :],
                            Ssb_flat[:, i, :],
                            start=(i == 0),
                            stop=(i == I - 1),
                        )
                    nc.scalar.copy(
                        out=sb_sraw[:, g_, :, 4 * nhalf : 4 * nhalf + 4, :]
                        .rearrange("o h n b -> o n h b"),
                        in_=sel_ps.rearrange("o n (h b) -> o n h b", h=BH),
                    )

                # ---- AllReduce (s_raw ++ Z) ----
                cc_in = dram.tile([CCLEN], F32, tag="cc_in")
                cc_out = dram.tile([CCLEN], F32, tag="cc_out")
                nc.gpsimd.dma_start(out=cc_in[0 : O * N * B], in_=sb_sraw)
                nc.gpsimd.dma_start(out=cc_in[O * N * B :], in_=sb_Z)
                if for_sim:
                    nc.gpsimd.dma_start(out=cc_out, in_=cc_in)
                else:
                    nc.gpsimd.collective_compute(
                        "AllReduce",
                        ALU.add,
                        replica_groups=[list(range(NC))],
                        ins=[cc_in.opt()],
                        outs=[cc_out.opt()],
                    )
                nc.gpsimd.dma_start(out=sb_sglob, in_=cc_out[0 : O * N * B])
                nc.gpsimd.dma_start(out=sb_Zg, in_=cc_out[O * N * B :])

                # ---- squash per (g,h) tile ----
                for t in range(NB_TILES):
                    g, h = t // BH, t % BH
                    s_sl = sb_sglob[:, 128 * t : 128 * (t + 1)]
                    sq_ps = ps_m.tile([128, 16], F32, tag="m")
                    nc.tensor.transpose(sq_ps, s_sl, sb_id16f)
                    sq_sb = sm.tile([128, 16], F32, tag="sqs")
                    nc.scalar.copy(out=sq_sb, in_=sq_ps)
                    sq2 = sm.tile([128, 16], F32, tag="sq2")
                    nc.vector.tensor_tensor(out=sq2, in0=sq_sb, in1=sq_sb, op=ALU.mult)
                    nc.vector.tensor_reduce(
                        out=sb_ss[:, t : t + 1], in_=sq2, axis=AX.X, op=ALU.add
                    )
                    if it == 3:
                        z2 = sm.tile([128, 1], F32, tag="z2")
                        nc.vector.tensor_tensor(
                            out=z2, in0=sb_Zg[:, t : t + 1], in1=sb_Zg[:, t : t + 1],
                            op=ALU.mult,
                        )
                        den = sm.tile([128, 1], F32, tag="den")
                        nc.vector.tensor_tensor(
                            out=den, in0=z2, in1=sb_ss[:, t : t + 1], op=ALU.add
                        )
                        rec = sm.tile([128, 1], F32, tag="rec")
                        nc.vector.reciprocal(out=rec, in_=den)
                        rss = sm.tile([128, 1], F32, tag="rss")
                        nc.scalar.sqrt(out=rss, in_=sb_ss[:, t : t + 1])
                        nc.vector.tensor_tensor(
                            out=sb_fac[:, t : t + 1], in0=rss, in1=rec, op=ALU.mult
                        )
                        vt = sm.tile([128, 16], F32, tag="vt")
                        nc.scalar.mul(out=vt, in_=sq_sb, mul=sb_fac[:, t : t + 1])
                        nc.sync.dma_start(
                            out=vout[8 * g : 8 * g + 8, 16 * h : 16 * h + 16, :],
                            in_=vt,
                        )
                if it < 3:
                    # batched squash scalars for all 10 tiles in one op each
                    z2a = sm.tile([128, NB_TILES], F32, tag="z2")
                    nc.vector.tensor_tensor(out=z2a, in0=sb_Zg, in1=sb_Zg, op=ALU.mult)
                    dena = sm.tile([128, NB_TILES], F32, tag="den")
                    nc.vector.tensor_tensor(out=dena, in0=z2a, in1=sb_ss, op=ALU.add)
                    reca = sm.tile([128, NB_TILES], F32, tag="rec")
                    nc.vector.reciprocal(out=reca, in_=dena)
                    rssa = sm.tile([128, NB_TILES], F32, tag="rss")
                    nc.scalar.sqrt(out=rssa, in_=sb_ss)
                    nc.vector.tensor_tensor(
                        out=sb_fac, in0=rssa, in1=reca, op=ALU.mult
                    )
                    # unnormalized v^T (the squash factor is applied later,
                    # per-partition, inside the wv PSUM->SBUF copy)
                    nc.scalar.copy(out=sb_vT, in_=sb_sglob)
                    for r in range(8):
                        nc.gpsimd.dma_start(
                            out=sb_vT8[16 * r : 16 * r + 16, :], in_=sb_vT
                        )

            def a_pass(it):
                """a[n,b,p] = u_hat . v for every local p; also fuses the
                bridge for the next s-pass (exp, Z, transpose into eP2)."""
                for g in range(NG):
                    bds, ats = [], []
                    for h in range(BH):
                        bd = sm.tile([128, 128], BF16, tag="bd", bufs=2)
                        t_ = 2 * g + h
                        nc.vector.tensor_tensor(
                            out=bd,
                            in0=sb_vT8[:, 128 * t_ : 128 * (t_ + 1)],
                            in1=sb_mask,
                            op=ALU.mult,
                        )
                        bds.append(bd)
                        ats.append(atile.tile([128, PL], F32, tag="a", name=f"at{h}"))

                    # stream w2[g] in 4 pieces of up to 256 nodes each
                    for pc in range(4):
                        pn = 256 if pc < 3 else 16
                        w2p = w2s.tile([128, 256, I], BF16, tag="w2p")
                        nc.sync.dma_start(
                            out=w2p[:, :pn, :],
                            in_=w2[g, :, 256 * pc : 256 * pc + pn, :],
                        )
                        for h in range(BH):
                            t = 2 * g + h
                            wv_sb = None
                            for sck in range(4 if pc < 3 else 1):
                                pw = 64 if pc < 3 else 16
                                F = pw * I
                                off = 256 * pc + 64 * sck  # global node offset
                                wv_ps = ps_wv.tile([128, 512], F32, tag="wv")
                                nc.tensor.matmul(
                                    wv_ps[:, :F],
                                    bds[h],
                                    w2p.rearrange("q p i -> q (p i)")[
                                        :, 512 * sck : 512 * sck + F
                                    ],
                                    start=True,
                                    stop=True,
                                )
                                # pair two 64-node chunks into one 128-node
                                # DVE fold to amortize per-op overhead
                                if wv_sb is None:
                                    wv_sb = work.tile([128, 128, I], BF16, tag="wvs")
                                half = sck % 2
                                nc.scalar.mul(
                                    out=wv_sb[:, 64 * half : 64 * half + pw, :],
                                    in_=wv_ps.rearrange("q (p i) -> q p i", i=I)[
                                        :, :pw, :
                                    ],
                                    mul=sb_fac[:, t : t + 1],
                                )
                                if pc < 3 and half == 0:
                                    continue  # wait for the second half
                                mw = 128 if pc < 3 else 16  # merged width
                                moff = off - 64 * half
                                ts_ = work.tile([128, 128, I], BF16, tag="ts")
                                nc.vector.tensor_tensor(
                                    out=ts_[:, :mw, :],
                                    in0=wv_sb[:, :mw, :],
                                    in1=sb_urep[h][:, moff : moff + mw, :],
                                    op=ALU.mult,
                                )
                                r1 = work.tile([128, 128, 4], BF16, tag="r1")
                                nc.vector.tensor_tensor(
                                    out=r1[:, :mw, :], in0=ts_[:, :mw, 0:4],
                                    in1=ts_[:, :mw, 4:8], op=ALU.add,
                                )
                                r2 = work.tile([128, 128, 2], BF16, tag="r2")
                                nc.vector.tensor_tensor(
                                    out=r2[:, :mw, :], in0=r1[:, :mw, 0:2],
                                    in1=r1[:, :mw, 2:4], op=ALU.add,
                                )
                                nc.vector.tensor_tensor(
                                    out=ats[h][:, moff : moff + mw],
                                    in0=r2[:, :mw, 0], in1=r2[:, :mw, 1], op=ALU.add,
                                )
                                wv_sb = None
                    # ---- bridge to next s-pass ----
                    for h in range(BH):
                        t = 2 * g + h
                        at = ats[h]
                        if it == 1:
                            nc.gpsimd.dma_start(out=a1_dram[t], in_=at)
                            bt = at
                        else:
                            a1b = atile.tile([128, PL], F32, tag="a1b")
                            nc.gpsimd.dma_start(out=a1b, in_=a1_dram[t])
                            bt = atile.tile([128, PL], F32, tag="b2")
                            nc.vector.tensor_tensor(out=bt, in0=at, in1=a1b, op=ALU.add)
                        e_nb = work.tile([128, PL], BF16, tag="enb")
                        nc.scalar.activation(
                            out=e_nb, in_=bt, func=ACTF.Exp,
                            accum_out=sb_Z[:, t : t + 1],
                        )
                        for ch in range(CH):
                            pw = 128 if ch < 6 else 16
                            eT_ps = ps_m.tile([128, 128], BF16, tag="m")
                            nc.tensor.transpose(
                                eT_ps[:pw, :],
                                e_nb[:, 128 * ch : 128 * ch + pw],
                                sb_id128b,
                            )
                            nc.scalar.copy(
                                out=sb_eP2[:pw, ch, 8 * g : 8 * g + 8,
                                           16 * h : 16 * h + 16],
                                in_=eT_ps.rearrange("p (n b) -> p n b", n=8)[:pw],
                            )

            s_pass(1)
            a_pass(1)
            s_pass(2)
            a_pass(2)
            s_pass(3)

    nc.finalize()
    return nc


_CACHE = {}


def _prep_inputs(u, W):
    """Per-core host-side relayout (not part of HW time)."""
    bf = ml_dtypes.bfloat16
    maps = []
    for c in range(NC):
        sl = slice(PL * c, PL * (c + 1))
        Wc = np.ascontiguousarray(W[:, sl])          # [40, 784, 8, 16] f32
        uc = np.ascontiguousarray(u[:, sl])          # [32, 784, 8] f32
        w2 = (
            Wc.reshape(NG, 8, PL, I, O)
            .transpose(0, 1, 4, 2, 3)
            .reshape(NG, 128, PL, I)
        )
        Wp = np.zeros((N, PPAD, I, O), np.float32)
        Wp[:, :PL] = Wc
        wp2 = Wp.reshape(N, CH, 128, 128).transpose(2, 1, 0, 3)
        Up = np.zeros((B, PPAD, I), np.float32)
        Up[:, :PL] = uc
        up2 = Up.reshape(B, CH, 128, I).transpose(2, 1, 3, 0)
        ur = np.broadcast_to(
            uc.reshape(1, BH, 16, PL, I), (8, BH, 16, PL, I)
        ).transpose(1, 0, 2, 3, 4).reshape(BH, 128, PL, I)
        bdm = np.zeros((128, 128), np.float32)
        for ns in range(8):
            bdm[ns * 16 : ns * 16 + 16, ns * 16 : ns * 16 + 16] = 1.0
        sel = np.zeros((128, I, 16), np.float32)
        for i in range(I):
            sel[16 * i : 16 * i + 16, i] = np.eye(16, dtype=np.float32)
        maps.append(
            {
                "w2": np.ascontiguousarray(w2).astype(bf),
                "wp2": np.ascontiguousarray(wp2).astype(bf),
                "up2": np.ascontiguousarray(up2).astype(bf),
                "urep": np.ascontiguousarray(ur).astype(bf),
                "bdmask": bdm.astype(bf),
                "selio": sel,
                "id128b": np.eye(128, dtype=np.float32).astype(bf),
                "id16f": np.eye(16, dtype=np.float32),
            }
        )
    return maps


def kernel(u, W):
    u = np.asarray(u, np.float32)
    W = np.asarray(W, np.float32)
    if "nc" not in _CACHE:
        _CACHE["nc"] = _build_program()
    nc = _CACHE["nc"]
    in_maps = _prep_inputs(u, W)
    res = run_bass_kernel_spmd(
        nc, in_maps, core_ids=list(range(NC)),
        trace=bool(int(os.environ.get("KERNEL_TRACE", "0"))),
    )
    _CACHE["last_result"] = res
    return res.results[0]["vout"]

